# revision 15
# baseline (speedup 1.0000x reference)
"""Trainium2 Bass kernel for nn_Adp (dense_cnn): 8-core SPMD.

Split: core c -> (sample s=c//2, half h=c%2). Odd cores get vertically
flipped slabs + H-flipped conv weights so all 8 cores run ONE program.
Host folds BN into conv weights and fuses the 2x bilinear upsample into
composite e1 weights (w' = 2 M w M^T). Convs = 9 shifted-tap fp32r
matmuls accumulated in PSUM; dynamic depthwise conv via on-device diag
matrices; encoder avg-pool completed with a pairwise AllReduce.

SBUF plan (192KB/part budget): big inputs (aux, mv) and the d1out
intermediate live in DRAM and are streamed per row-tile; three rotating
SBUF slab slots: S1 mv_aux->e1o(d2), S2 e1o(d1)->c1out->d2out, SC dwo.
"""

import numpy as np
from contextlib import ExitStack

import concourse.bass as bass
import concourse.tile as tile
from concourse import bacc
from concourse import mybir
from concourse import bass_utils

F32 = mybir.dt.float32
F32R = mybir.dt.bfloat16  # matmul compute dtype
I32 = mybir.dt.int32

B, C, H, W = 4, 64, 128, 128
PW = W + 2  # padded width 130

R_AUX_IN = 71   # input rows 0..70 (+1 top pad row -> 72)
R_MVAUX = 70    # aw out rows 0..69 (+1 top pad -> 71)
R_E1 = 67       # e1 out rows 0..66
R_E2 = 33       # e2 out rows 0..32
R_E3 = 31       # e3 out rows 0..30 (row 30 pooled with weight 0.5)
R_DW1 = 69      # d1 depthwise out rows 0..68
R_C1 = 68       # c1 out rows 0..67 (+1 pad -> 69)
R_MV_IN = 66    # mv input rows 0..65 (+1 pad -> 67)
R_DW2 = 65      # d2 depthwise out rows 0..64
R_D2 = 65       # d2out rows 0..64 (+1 pad -> 66)
R_OUT = 64      # final rows 0..63

RG = [[0, 1], [2, 3], [4, 5], [6, 7]]
LR = mybir.ActivationFunctionType.Lrelu
RELU = mybir.ActivationFunctionType.Relu
IDN = mybir.ActivationFunctionType.Identity
ADD = mybir.AluOpType.add
MUL = mybir.AluOpType.mult

_CACHED = {}

import copy as _copy
import dataclasses as _dc


def _r(ap):
    """Matmul operand tiles are declared float32r directly; no-op."""
    return ap



# ---------------------------------------------------------------- host prep

def _fold_bn(w, b, g, beta, m, v):
    s = g / np.sqrt(v + 1e-5)
    return w * s[:, None, None, None], (b - m) * s + beta


def _pack(w, flip):
    # w [co, ci, 3, 3] -> lhsT pack [ci, 9*co], tap t=dy*3+dx slice [ci, co]
    if flip:
        w = w[:, :, ::-1, :]
    co, ci = w.shape[0], w.shape[1]
    return np.ascontiguousarray(w.transpose(1, 2, 3, 0).reshape(ci, 9 * co)).astype(np.float32)


_M = np.array([[.75, .25, 0], [.25, .75, .75], [0, 0, .25]], np.float32)


def _prep_weights(p, flip):
    d = {}
    d['w_aw'] = _pack(np.asarray(p['aw'], np.float32), flip)
    d['b_aw'] = np.asarray(p['ab'], np.float32).reshape(C, 1)
    for pre in ('d1_', 'd2_'):
        P = lambda k: np.asarray(p[pre + k], np.float32)
        w1, b1 = _fold_bn(P('e1w'), P('e1b'), P('bn1g'), P('bn1b'), P('bn1m'), P('bn1v'))
        wc = 2.0 * np.einsum('xa,yb,ocab->ocxy', _M, _M, w1).astype(np.float32)
        d[pre + 'w_e1'] = _pack(wc, flip)
        d[pre + 'b_e1'] = b1.reshape(128, 1)
        w2, b2 = _fold_bn(P('e2w'), P('e2b'), P('bn2g'), P('bn2b'), P('bn2m'), P('bn2v'))
        d[pre + 'w_e2'] = _pack(w2, flip)
        d[pre + 'b_e2'] = b2.reshape(128, 1)
        w3, b3 = _fold_bn(P('e3w'), P('e3b'), P('bn3g'), P('bn3b'), P('bn3m'), P('bn3v'))
        d[pre + 'w_e3a'] = _pack(w3[:128], flip)
        d[pre + 'w_e3b'] = _pack(w3[128:], flip)
        d[pre + 'b_e3'] = b3.reshape(2, 128).T.copy()
        m1 = P('m1w') / 3721.0
        d[pre + 'm1wT'] = np.concatenate([m1.T[:128], m1.T[128:]], axis=1)  # [128,512]
        d[pre + 'm1b'] = P('m1b').reshape(2, 128).T.copy()
        m2 = P('m2w')
        d[pre + 'm2wT'] = np.concatenate([m2.T[:128], m2.T[128:]], axis=1)
        d[pre + 'm2b'] = P('m2b').reshape(2, 128).T.copy()
        k1 = P('kw1')
        d[pre + 'kw1T'] = np.concatenate([k1.T[:128], k1.T[128:]], axis=1)  # [128,256]
        k2 = P('kw2')  # [576, 128]
        perm = np.empty(576, np.int64)
        for t in range(9):
            dy, dx = t // 3, t % 3
            tp = (3 * (2 - dy) + dx) if flip else t
            for c in range(C):
                perm[t * C + c] = c * 9 + tp
        d[pre + 'kw2T'] = np.ascontiguousarray(k2[perm].T)  # [128, 576]
        c1_ = P('ca1')
        d[pre + 'ca1T'] = np.concatenate([c1_.T[:128], c1_.T[128:]], axis=1)  # [128,64]
        d[pre + 'ca2T'] = np.ascontiguousarray(P('ca2').T)  # [32, 64]
        d[pre + 'cwT'] = np.ascontiguousarray(P('cw').reshape(C, C).T)
        d[pre + 'cb'] = P('cb').reshape(C, 1)
    d['w_c1'] = _pack(np.asarray(p['c1w'], np.float32), flip)
    d['b_c1'] = np.asarray(p['c1b'], np.float32).reshape(C, 1)
    d['w_c2'] = _pack(np.asarray(p['c2w'], np.float32), flip)
    d['b_c2'] = np.asarray(p['c2b'], np.float32).reshape(C, 1)
    return d


def _slab(img, nrows):
    c = img.shape[0]
    s = np.zeros((c, 1 + nrows, PW), np.float32)
    s[:, 1:, 1:129] = img[:, :nrows, :]
    return s


def _prep_core(mv, a2, a3, p, s, h):
    flip = (h == 1)
    sl = (lambda a: np.asarray(a, np.float32)[s, :, ::-1, :] if flip
          else np.asarray(a, np.float32)[s])
    d = _prep_weights(p, flip)
    aux = np.concatenate([sl(a2), sl(a3)], axis=0)
    d['aux'] = _slab(aux, R_AUX_IN)
    d['mv'] = _slab(sl(mv), R_MV_IN)
    return d


# ---------------------------------------------------------------- device

def _build_program():
    nc = bacc.Bacc("TRN2", target_bir_lowering=False, debug=False,
                   num_devices=8)

    def din(name, shape, dt=F32):
        return nc.declare_dram_parameter(name, list(shape), dt, isOutput=False)

    aux_d = din('aux', [128, 1 + R_AUX_IN, PW])
    mv_d = din('mv', [C, 1 + R_MV_IN, PW])
    wd, bd = {}, {}
    for nm, shp in [('w_aw', (128, 9 * C)), ('w_c1', (C, 9 * C)), ('w_c2', (C, 9 * C))]:
        wd[nm] = din(nm, shp)
    for nm in ('b_aw', 'b_c1', 'b_c2'):
        bd[nm] = din(nm, (C, 1), F32)
    for pre in ('d1_', 'd2_'):
        for nm, shp in [('w_e1', (C, 9 * 128)), ('w_e2', (128, 9 * 128)),
                        ('w_e3a', (128, 9 * 128)), ('w_e3b', (128, 9 * 128)),
                        ('cwT', (C, C)), ('m1wT', (128, 512)), ('m2wT', (128, 512)),
                        ('kw1T', (128, 256)), ('kw2T', (128, 576)),
                        ('ca1T', (128, 64)), ('ca2T', (32, 64))]:
            wd[pre + nm] = din(pre + nm, shp)
        for nm, shp in [('b_e1', (128, 1)), ('b_e2', (128, 1)), ('b_e3', (128, 2)),
                        ('m1b', (128, 2)), ('m2b', (128, 2)), ('cb', (C, 1))]:
            bd[pre + nm] = din(pre + nm, shp, F32)
    out_d = nc.declare_dram_parameter('out', [C, R_OUT, W], F32, isOutput=True)

    d1o_d = nc.dram_tensor('d1o_dram', [C, R_DW1, W], F32R)
    cc_in = [nc.dram_tensor(f'cc_in{i}', [2, 128], F32) for i in range(2)]
    cc_out = [nc.dram_tensor(f'cc_out{i}', [2, 128], F32) for i in range(2)]

    with tile.TileContext(nc) as tc, ExitStack() as ctx:
        ep = ctx.enter_context
        # persistent weights (small)
        wp = ep(tc.tile_pool(name="wp", bufs=1))
        # shared-slot encoder/mlp weights (d1 then d2 reuse the same slot)
        we = ep(tc.tile_pool(name="we", bufs=1))
        bp = ep(tc.tile_pool(name="bp", bufs=1))
        small = ep(tc.tile_pool(name="small", bufs=1))
        diagp = ep(tc.tile_pool(name="diag", bufs=1))
        pS1 = ep(tc.tile_pool(name="S1", bufs=1))
        pS2 = ep(tc.tile_pool(name="S2", bufs=1))
        pSC = ep(tc.tile_pool(name="SC", bufs=1))
        pE2 = ep(tc.tile_pool(name="E2", bufs=1))
        strm = ep(tc.tile_pool(name="strm", bufs=2))
        psp = ep(tc.tile_pool(name="psp", bufs=4, space="PSUM"))
        pst = ep(tc.tile_pool(name="pst", bufs=2, space="PSUM"))
        tmpp = ep(tc.tile_pool(name="tmp", bufs=2))

        wt = {}
        for nm in ('w_aw', 'w_c1', 'w_c2'):
            t = wp.tile(list(wd[nm].shape), F32R, tag=nm)
            nc.gpsimd.dma_start(t[:], wd[nm].ap())
            wt[nm] = t
        bt = {}
        for nm, dram in bd.items():
            t = bp.tile(list(dram.shape), F32, tag=nm)
            nc.gpsimd.dma_start(t[:], dram.ap())
            bt[nm] = t

        def enc_w(pre):
            """Load this da-block's encoder/MLP weights into the shared slot."""
            out = {}
            for nm in ('w_e1', 'w_e2', 'cwT', 'm1wT', 'm2wT',
                       'kw1T', 'kw2T', 'ca1T', 'ca2T'):
                t = we.tile(list(wd[pre + nm].shape), F32R, tag=nm)
                nc.gpsimd.dma_start(t[:], wd[pre + nm].ap())
                out[nm] = t
            return out

        # diag mask
        ic = small.tile([C, C], I32, tag='ic')
        ipt = small.tile([C, C], I32, tag='ip')
        nc.gpsimd.iota(ic[:], pattern=[[1, C]], base=0, channel_multiplier=0)
        nc.gpsimd.iota(ipt[:], pattern=[[0, C]], base=0, channel_multiplier=1)
        mask = small.tile([C, C], F32, tag='mask')
        nc.vector.tensor_tensor(out=mask[:], in0=ic[:], in1=ipt[:],
                                op=mybir.AluOpType.is_equal)

        def pad_slab(t):
            nc.gpsimd.memset(t[:, 0, :], 0.0)
            nc.gpsimd.memset(t[:, :, 0:PW:PW - 1], 0.0)

        def conv_pad(src, wtile, co, nrows, out_fn, tag):
            """3x3 pad=1 conv from padded SBUF slab src."""
            r0 = 0
            while r0 < nrows:
                nr = min(4, nrows - r0)
                ps = psp.tile([co, nr, 128], F32, tag='ps')
                for t in range(9):
                    dy, dx = t // 3, t % 3
                    nc.tensor.matmul(ps[:], _r(wtile[:, t * co:(t + 1) * co]),
                                     _r(src[:, r0 + dy:r0 + dy + nr, dx:dx + 128]),
                                     start=(t == 0), stop=(t == 8))
                out_fn(r0, nr, ps)
                r0 += nr

        def conv_stream(dram_ap, nin, ci, wtile, co, nrows, out_fn, tag,
                        from_unpadded=False):
            """3x3 pad=1 conv streaming its (host- or device-padded) input
            from DRAM. dram_ap rows: padded slab [ci, 1+nin, PW] unless
            from_unpadded ([ci, nin, W] -> pad cols/top on device)."""
            r0 = 0
            while r0 < nrows:
                nr = min(4, nrows - r0)
                mini = strm.tile([ci, 6, PW], F32R, tag='m' + tag)
                if not from_unpadded:
                    nc.gpsimd.dma_start(mini[:, 0:nr + 2, :], dram_ap[:, r0:r0 + nr + 2, :])
                else:
                    nc.gpsimd.memset(mini[:, 0:nr + 2, 0:PW:PW - 1], 0.0)
                    if r0 == 0:
                        nc.gpsimd.memset(mini[:, 0, :], 0.0)
                        nc.gpsimd.dma_start(mini[:, 1:nr + 2, 1:129],
                                          dram_ap[:, 0:nr + 1, :])
                    else:
                        nc.gpsimd.dma_start(mini[:, 0:nr + 2, 1:129],
                                          dram_ap[:, r0 - 1:r0 + nr + 1, :])
                ps = psp.tile([co, nr, 128], F32, tag='ps')
                for t in range(9):
                    dy, dx = t // 3, t % 3
                    nc.tensor.matmul(ps[:], _r(wtile[:, t * co:(t + 1) * co]),
                                     _r(mini[:, dy:dy + nr, dx:dx + 128]),
                                     start=(t == 0), stop=(t == 8))
                out_fn(r0, nr, ps, mini)
                r0 += nr

        # --- aw conv: aux (streamed) -> mv_aux slab (lrelu 0.01)
        mv_aux = pS1.tile([C, 1 + R_MVAUX, PW], F32R, tag='S1')
        pad_slab(mv_aux)

        def aw_out(r0, nr, ps, mini):
            nc.scalar.activation(mv_aux[:, 1 + r0:1 + r0 + nr, 1:129], ps[:],
                                 LR, bias=bt['b_aw'][:], alpha=0.01)
        conv_stream(aux_d.ap(), R_AUX_IN, 128, wt['w_aw'], C, R_MVAUX, aw_out, 'aw')

        sem_d = [nc.alloc_semaphore(f'ar_d{i}') for i in range(2)]
        sem_c = [nc.alloc_semaphore(f'ar_c{i}') for i in range(2)]

        def encoder_block(idx, pre, ew, ysrc, e1pool, e1tag):
            # e1' composite conv
            e1o = e1pool.tile([128, R_E1, 128], F32R, tag=e1tag)

            def e1_out(r0, nr, ps):
                nc.scalar.activation(e1o[:, r0:r0 + nr, :], ps[:], LR,
                                     bias=bt[pre + 'b_e1'][:], alpha=0.1)
            conv_pad(ysrc, ew['w_e1'], 128, R_E1, e1_out, 'e1')

            # e2 stride-2 valid conv
            e2o = pE2.tile([128, R_E2, 63], F32R, tag='E2')
            r0 = 0
            while r0 < R_E2:
                nr = min(8, R_E2 - r0)
                ps = psp.tile([128, nr, 63], F32, tag='ps')
                for t in range(9):
                    dy, dx = t // 3, t % 3
                    nc.tensor.matmul(ps[:], _r(ew['w_e2'][:, t * 128:(t + 1) * 128]),
                        _r(e1o[:, 2 * r0 + dy:2 * r0 + dy + 2 * nr - 1:2, dx:dx + 126:2]),
                        start=(t == 0), stop=(t == 8))
                nc.scalar.activation(e2o[:, r0:r0 + nr, :], ps[:], LR,
                                     bias=bt[pre + 'b_e2'][:], alpha=0.1)
                r0 += nr

            # e3 valid conv, 2 co chunks; lrelu in-place on PSUM, pool there
            sums = small.tile([128, 2], F32, tag=f'sums{idx}')
            r30 = small.tile([128, 2], F32, tag=f'r30{idx}')
            for ck in range(2):
                wch = we.tile([128, 9 * 128], F32R, tag='w_e3')
                nc.gpsimd.dma_start(wch[:], wd[pre + ('w_e3a' if ck == 0 else 'w_e3b')].ap())
                r0 = 0
                while r0 < R_E3:
                    nr = min(8, R_E3 - r0)
                    ps = psp.tile([128, nr, 61], F32, tag='ps')
                    for t in range(9):
                        dy, dx = t // 3, t % 3
                        nc.tensor.matmul(ps[:], _r(wch[:, t * 128:(t + 1) * 128]),
                                         _r(e3o_rhs(e2o, r0, nr, dy, dx)),
                                         start=(t == 0), stop=(t == 8))
                    nc.scalar.activation(ps[:], ps[:], LR,
                                         bias=bt[pre + 'b_e3'][:, ck:ck + 1], alpha=0.1)
                    red = small.tile([128, 1], F32, tag='red')
                    nc.vector.tensor_reduce(red[:], ps[:],
                                            axis=mybir.AxisListType.XY, op=ADD)
                    if r0 == 0:
                        nc.vector.tensor_copy(sums[:, ck:ck + 1], red[:])
                    else:
                        nc.vector.tensor_tensor(out=sums[:, ck:ck + 1],
                                                in0=sums[:, ck:ck + 1], in1=red[:], op=ADD)
                    if r0 + nr == R_E3:
                        nc.vector.tensor_reduce(r30[:, ck:ck + 1], ps[:, nr - 1, :],
                                                axis=mybir.AxisListType.X, op=ADD)
                    r0 += nr
            part = small.tile([128, 2], F32, tag=f'part{idx}')
            nc.vector.tensor_scalar(out=r30[:], in0=r30[:], scalar1=0.5,
                                    scalar2=None, op0=MUL)
            nc.vector.tensor_tensor(out=part[:], in0=sums[:], in1=r30[:],
                                    op=mybir.AluOpType.subtract)

            fea = small.tile([128, 2], F32R, tag=f'fea{idx}')
            with tc.tile_critical():
                g = nc.gpsimd
                for ck in range(2):
                    g.dma_start(out=cc_in[idx].ap()[ck:ck + 1, :],
                                in_=part[:, ck:ck + 1]).then_inc(sem_d[idx], 16)
                g.wait_ge(sem_d[idx], 32)
                g.collective_compute(
                    "AllReduce", ADD, replica_groups=RG,
                    ins=[cc_in[idx].ap().opt()], outs=[cc_out[idx].ap().opt()],
                ).then_inc(sem_c[idx], 1)
                g.wait_ge(sem_c[idx], 1)
                for ck in range(2):
                    g.dma_start(out=fea[:, ck:ck + 1],
                                in_=cc_out[idx].ap()[ck:ck + 1, :]).then_inc(sem_d[idx], 16)
                g.wait_ge(sem_d[idx], 64)

            def vmm(lhs_list, rhs_list, M, bias, act, alpha, outtag, dt=F32R):
                o = small.tile([M, 1], dt, tag=outtag)
                ps = pst.tile([M, 1], F32, tag='vs')
                nK = len(lhs_list)
                for ki in range(nK):
                    nc.tensor.matmul(ps[:], _r(lhs_list[ki]), _r(rhs_list[ki]),
                                     start=(ki == 0), stop=(ki == nK - 1))
                if act is None:
                    nc.scalar.activation(o[:], ps[:], IDN, bias=bias)
                else:
                    nc.scalar.activation(o[:], ps[:], act, bias=bias, alpha=alpha)
                return o

            m1w, m2w = ew['m1wT'], ew['m2wT']
            h1 = [vmm([m1w[:, ki * 256 + mo * 128: ki * 256 + (mo + 1) * 128] for ki in range(2)],
                      [fea[:, 0:1], fea[:, 1:2]], 128,
                      bt[pre + 'm1b'][:, mo:mo + 1], LR, 0.1, f'h1_{mo}')
                  for mo in range(2)]
            rep = [vmm([m2w[:, ki * 256 + mo * 128: ki * 256 + (mo + 1) * 128] for ki in range(2)],
                       [h1[0][:], h1[1][:]], 128,
                       bt[pre + 'm2b'][:, mo:mo + 1], None, 0.0, f'rep_{mo}')
                   for mo in range(2)]
            k1 = vmm([ew['kw1T'][:, ki * 128:(ki + 1) * 128] for ki in range(2)],
                     [rep[0][:], rep[1][:]], 128, 0.0, LR, 0.1, 'k1')
            kv = small.tile([128, 5], F32, tag='kv')
            for j in range(5):
                mj = 128 if j < 4 else 64
                ps = pst.tile([mj, 1], F32, tag='vs')
                nc.tensor.matmul(ps[:], _r(ew['kw2T'][:, j * 128:j * 128 + mj]), _r(k1[:]),
                                 start=True, stop=True)
                nc.scalar.activation(kv[0:mj, j:j + 1], ps[:], IDN)
            a1 = vmm([ew['ca1T'][:, ki * 32:(ki + 1) * 32] for ki in range(2)],
                     [rep[0][:], rep[1][:]], 32, 0.0, LR, 0.1, 'a1')
            att = vmm([ew['ca2T'][:]], [a1[:]], 64, 0.0, RELU, 0.0, f'att_{idx}', dt=F32)
            diags = []
            for t in range(9):
                dg = diagp.tile([C, C], F32R, tag=f'diag{idx}_{t}')
                nc.vector.tensor_scalar(
                    out=dg[:], in0=mask[:],
                    scalar1=kv[(t % 2) * 64:(t % 2) * 64 + 64, t // 2:t // 2 + 1],
                    scalar2=None, op0=MUL)
                diags.append(dg)
            return diags, att

        def e3o_rhs(e2o, r0, nr, dy, dx):
            return e2o[:, r0 + dy:r0 + dy + nr, dx:dx + 61]

        def dw_cw(idx, diags, att, cwT, cb, r_dw, mini_fn, rhs_fn, x_fn, out_fn):
            """depthwise(lrelu .1) -> cw 1x1 + cb + x*att -> out_fn(r0,nr,ps)."""
            dwo = pSC.tile([C, r_dw, 128], F32R, tag='SC')
            r0 = 0
            while r0 < r_dw:
                nr = min(4, r_dw - r0)
                mini = mini_fn(r0, nr)
                ps = psp.tile([C, nr, 128], F32, tag='ps')
                for t in range(9):
                    dy, dx = t // 3, t % 3
                    nc.tensor.matmul(ps[:], _r(diags[t][:]),
                                     _r(rhs_fn(mini, r0, nr, dy, dx)),
                                     start=(t == 0), stop=(t == 8))
                nc.scalar.activation(dwo[:, r0:r0 + nr, :], ps[:], LR, alpha=0.1)
                r0 += nr
            r0 = 0
            while r0 < r_dw:
                nr = min(4, r_dw - r0)
                ps = psp.tile([C, nr, 128], F32, tag='ps')
                nc.tensor.matmul(ps[:], _r(cwT[:]), _r(dwo[:, r0:r0 + nr, :]),
                                 start=True, stop=True)
                t1 = tmpp.tile([C, 4, 128], F32, tag='t1')
                nc.vector.tensor_scalar(out=t1[:, 0:nr, :], in0=x_fn(r0, nr),
                                        scalar1=att[:], scalar2=cb[:],
                                        op0=MUL, op1=ADD)
                nc.vector.tensor_tensor(out=ps[:], in0=ps[:], in1=t1[:, 0:nr, :], op=ADD)
                out_fn(r0, nr, ps)
                r0 += nr

        # ================= d1 =================
        ew1 = enc_w('d1_')
        diags1, att1 = encoder_block(0, 'd1_', ew1, mv_aux, pS2, 'S2')

        def d1_out(r0, nr, ps):
            ob = tmpp.tile([C, 4, 128], F32R, tag='ob')
            nc.scalar.activation(ob[:, 0:nr, :], ps[:], LR, alpha=0.1)
            nc.gpsimd.dma_start(d1o_d.ap()[:, r0:r0 + nr, :], ob[:, 0:nr, :])

        dw_cw(0, diags1, att1, ew1['cwT'], bt['d1_cb'], R_DW1,
              lambda r0, nr: None,
              lambda m, r0, nr, dy, dx: mv_aux[:, r0 + dy:r0 + dy + nr, dx:dx + 128],
              lambda r0, nr: mv_aux[:, 1 + r0:1 + r0 + nr, 1:129],
              d1_out)

        # ================= c1 (streams d1o from DRAM) =================
        c1o = pS2.tile([C, 1 + R_C1, PW], F32R, tag='S2')
        pad_slab(c1o)

        def c1_out(r0, nr, ps, mini):
            nc.scalar.activation(c1o[:, 1 + r0:1 + r0 + nr, 1:129], ps[:], LR,
                                 bias=bt['b_c1'][:], alpha=0.1)
        conv_stream(d1o_d.ap(), R_DW1, C, wt['w_c1'], C, R_C1, c1_out, 'c1',
                    from_unpadded=True)

        # ================= d2 (x = mv, streamed) =================
        ew2 = enc_w('d2_')
        diags2, att2 = encoder_block(1, 'd2_', ew2, c1o, pS1, 'S1')

        d2o = pS2.tile([C, 1 + R_D2, PW], F32R, tag='S2')
        pad_slab(d2o)

        def mv_mini(r0, nr):
            m = strm.tile([C, 6, PW], F32R, tag='mmv')
            nc.gpsimd.dma_start(m[:, 0:nr + 2, :], mv_d.ap()[:, r0:r0 + nr + 2, :])
            return m

        def mv_int(r0, nr):
            m = strm.tile([C, 4, 128], F32, tag='mvr')
            nc.gpsimd.dma_start(m[:, 0:nr, :], mv_d.ap()[:, 1 + r0:1 + r0 + nr, 1:129])
            return m[:, 0:nr, :]

        def d2_out(r0, nr, ps):
            nc.scalar.activation(d2o[:, 1 + r0:1 + r0 + nr, 1:129], ps[:], LR,
                                 alpha=0.1)

        dw_cw(1, diags2, att2, ew2['cwT'], bt['d2_cb'], R_DW2,
              mv_mini,
              lambda m, r0, nr, dy, dx: m[:, dy:dy + nr, dx:dx + 128],
              mv_int,
              d2_out)

        # ================= c2 + mv residual -> out =================
        r0 = 0
        while r0 < R_OUT:
            nr = 4
            ps = psp.tile([C, nr, 128], F32, tag='ps')
            for t in range(9):
                dy, dx = t // 3, t % 3
                nc.tensor.matmul(ps[:], _r(wt['w_c2'][:, t * C:(t + 1) * C]),
                                 _r(d2o[:, r0 + dy:r0 + dy + nr, dx:dx + 128]),
                                 start=(t == 0), stop=(t == 8))
            mvr = strm.tile([C, 4, 128], F32, tag='mvr')
            nc.gpsimd.dma_start(mvr[:], mv_d.ap()[:, 1 + r0:1 + r0 + nr, 1:129])
            nc.vector.tensor_tensor(out=ps[:], in0=ps[:], in1=mvr[:], op=ADD)
            ob = tmpp.tile([C, 4, 128], F32, tag='ob2')
            nc.scalar.activation(ob[:], ps[:], IDN, bias=bt['b_c2'][:])
            nc.gpsimd.dma_start(out_d.ap()[:, r0:r0 + nr, :], ob[:])
            r0 += nr

    nc.compile()
    return nc


# ---------------------------------------------------------------- entry

def kernel(mv, mv_aux2, mv_aux3, params):
    if 'nc' not in _CACHED:
        _CACHED['nc'] = _build_program()
    nc = _CACHED['nc']
    in_maps = [_prep_core(mv, mv_aux2, mv_aux3, params, c // 2, c % 2)
               for c in range(8)]
    res = bass_utils.run_bass_kernel_spmd(nc, in_maps, core_ids=list(range(8)))
    out = np.empty((B, C, H, W), np.float32)
    for s in range(B):
        out[s, :, 0:64, :] = res.results[2 * s]['out']
        out[s, :, 64:128, :] = res.results[2 * s + 1]['out'][:, ::-1, :]
    return out
